# revision 7
# baseline (speedup 1.0000x reference)
"""DeepFM forward on 8 Trainium2 NeuronCores (Bass/Tile).

Sharding: data-parallel over the 16384 batch (2048 rows/core); all embedding
tables are replicated per core as one combined [V_tot, 33] table
(32 fm2/emb cols + 1 fm1 col). Per core the kernel:
  1. indirect-DMA gathers 9 field rows per batch element (batch-major G),
  2. computes the FM 1st/2nd-order terms batch-major on DVE/ACT,
  3. transposes gathered features via PE matmuls-with-identity into a
     feature-major dnn input, runs the 3-layer MLP feature-major on PE,
  4. combines everything batch-major and writes [2048] f32 per core.
"""
import sys

sys.path.insert(0, "/opt/trn_rl_repo")

import numpy as np

import concourse.bacc as bacc
import concourse.bass as bass
import concourse.tile as tile
from concourse import mybir
from concourse import bass_utils
from concourse.masks import make_identity

# ---------------------------------------------------------------- constants
B = 16384
NCORES = 8
BC = B // NCORES          # 2048 batch rows per core
P = 128
NT = BC // P              # 16 batch tiles per core
EMB = 32
D = EMB + 1               # 33: fm2 row + fm1 col

# gather field order: 4 item cate, 3 user cate, item_emb, user_emb
ITEM_CATE_COLS = [4, 5, 6, 7]
USER_CATE_COLS = [2, 3, 4]
VOCS = [100000, 50000, 10000, 1000, 200000, 50000, 5000, 500000, 1000000]
NF = len(VOCS)            # 9 gathered fields
BASES = np.concatenate([[0], np.cumsum(VOCS)[:-1]]).astype(np.int64)
VTOT = int(np.sum(VOCS))  # 1,916,000

# dnn field order (12 fields x 32): item cate x4, item oh x2, user cate x3,
# user oh x1, item_emb, user_emb
DNN_FIELD_OF_GATHER = [0, 1, 2, 3, 6, 7, 8, 10, 11]  # dnn field idx per gather field
DNN_FIELD_OF_OH = [4, 5, 9]

GW = 297                  # gathered block: 9 fields * 33
OHW = 97                  # onehot matmul out: 96 E_oh + 1 fm1_xd
GBLK = 400                # per-tile block stride in G (297 + 97 + 6 pad)
NK = 4                    # feature k-chunks of the 393-row layout
KSIZES = [128, 128, 128, 9]
KOFF = [0, 128, 256, 384]
NN = 4                    # n-chunks (batch 2048 / 512)
NB = 512
EPS = 1e-5
F32 = mybir.dt.float32
I32 = mybir.dt.int32

_cache = {}


def _build():
    """Build + compile the per-core Bass kernel (identical on all cores)."""
    nc = bacc.Bacc(
        "TRN2", target_bir_lowering=False, debug=False,
        enable_asserts=False, num_devices=NCORES,
    )
    tbl = nc.dram_tensor("tbl", [VTOT, D], F32, kind="ExternalInput").ap()
    idx = nc.dram_tensor("idx", [P, NT * NF], I32, kind="ExternalInput").ap()
    xdt = nc.dram_tensor("xdt", [46, BC], F32, kind="ExternalInput").ap()
    wcat = nc.dram_tensor("wcat", [46, OHW], F32, kind="ExternalInput").ap()
    w0p = nc.dram_tensor("w0p", [393, 256], F32, kind="ExternalInput").ap()
    dwp = nc.dram_tensor("dwp", [6, 393], F32, kind="ExternalInput").ap()
    dbp = nc.dram_tensor("dbp", [393, 1], F32, kind="ExternalInput").ap()
    b0f = nc.dram_tensor("b0f", [256, 1], F32, kind="ExternalInput").ap()
    w1p = nc.dram_tensor("w1p", [256, 128], F32, kind="ExternalInput").ap()
    b1f = nc.dram_tensor("b1f", [128, 1], F32, kind="ExternalInput").ap()
    w2 = nc.dram_tensor("w2", [128, 1], F32, kind="ExternalInput").ap()
    cb = nc.dram_tensor("cb", [1, 1], F32, kind="ExternalInput").ap()  # fm1_dense_b
    dft = nc.dram_tensor("dft", [6, BC], F32, kind="ExternalInput").ap()
    out = nc.dram_tensor("out", [BC], F32, kind="ExternalOutput").ap()

    with tile.TileContext(nc) as tc:
        _body(nc, tc, tbl, idx, xdt, wcat, w0p, dwp, dbp, b0f, w1p, b1f, w2, cb, dft, out)
    nc.compile()
    return nc


def _body(nc, tc, tbl, idx, xdt, wcat, w0p, dwp, dbp, b0f, w1p, b1f, w2, cb, dft, out):
    import contextlib
    ctx = contextlib.ExitStack()
    with ctx:
        big = ctx.enter_context(tc.tile_pool(name="big", bufs=1))
        wpool = ctx.enter_context(tc.tile_pool(name="wpool", bufs=1))
        work = ctx.enter_context(tc.tile_pool(name="work", bufs=2))
        ps_oh = ctx.enter_context(tc.tile_pool(name="ps_oh", bufs=2, space="PSUM"))
        ps_tr = ctx.enter_context(tc.tile_pool(name="ps_tr", bufs=2, space="PSUM"))
        ps_mm = ctx.enter_context(tc.tile_pool(name="ps_mm", bufs=2, space="PSUM"))
        ps_dn = ctx.enter_context(tc.tile_pool(name="ps_dn", bufs=1, space="PSUM"))

        # ---------------- load inputs to SBUF
        idx_t = wpool.tile([P, NT * NF], I32)
        nc.sync.dma_start(out=idx_t[:], in_=idx)
        xdt_t = wpool.tile([46, BC], F32)
        nc.sync.dma_start(out=xdt_t[:], in_=xdt)
        wcat_t = wpool.tile([46, OHW], F32)
        nc.sync.dma_start(out=wcat_t[:], in_=wcat)
        w0_t = [wpool.tile([KSIZES[k], 256], F32, name=f"w0t{k}", tag=f"w0_{k}") for k in range(NK)]
        for k in range(NK):
            nc.sync.dma_start(out=w0_t[k][:], in_=w0p[KOFF[k]:KOFF[k] + KSIZES[k], :])
        dwp_t = wpool.tile([6, 393], F32)
        nc.sync.dma_start(out=dwp_t[:], in_=dwp)
        dbp_t = [wpool.tile([KSIZES[k], 1], F32, name=f"dbt{k}", tag=f"db_{k}") for k in range(NK)]
        for k in range(NK):
            nc.sync.dma_start(out=dbp_t[k][:], in_=dbp[KOFF[k]:KOFF[k] + KSIZES[k], :])
        b0_t = wpool.tile([P, 2], F32)
        nc.sync.dma_start(out=b0_t[:, 0:1], in_=b0f[0:128, :])
        nc.sync.dma_start(out=b0_t[:, 1:2], in_=b0f[128:256, :])
        w1_t = [wpool.tile([P, P], F32, name=f"w1t{m}", tag=f"w1_{m}") for m in range(2)]
        for m in range(2):
            nc.sync.dma_start(out=w1_t[m][:], in_=w1p[m * P:(m + 1) * P, :])
        b1_t = wpool.tile([128, 1], F32)
        nc.sync.dma_start(out=b1_t[:], in_=b1f)
        w2_t = wpool.tile([128, 1], F32)
        nc.sync.dma_start(out=w2_t[:], in_=w2)
        cb_t = wpool.tile([P, 1], F32)
        nc.gpsimd.dma_start(out=cb_t[:], in_=cb.to_broadcast([P, 1]))
        dft_t = wpool.tile([6, BC], F32)
        nc.sync.dma_start(out=dft_t[:], in_=dft)
        ident = wpool.tile([P, P], F32)
        make_identity(nc, ident[:])

        # ---------------- gather: G[p, t, 0:297] = tbl rows for batch t*128+p
        G = big.tile([P, NT, GBLK], F32)
        for t in range(NT):
            for f in range(NF):
                nc.gpsimd.indirect_dma_start(
                    out=G[:, t, f * D:(f + 1) * D],
                    out_offset=None,
                    in_=tbl,
                    in_offset=bass.IndirectOffsetOnAxis(
                        ap=idx_t[:, t * NF + f:t * NF + f + 1], axis=0),
                )

        # ---------------- onehot + fm1-dense matmul: per tile [128,97] psum
        for t in range(NT):
            oh_ps = ps_oh.tile([P, OHW], F32, tag="oh")
            nc.tensor.matmul(oh_ps[:], lhsT=xdt_t[:, t * P:(t + 1) * P],
                             rhs=wcat_t[:], start=True, stop=True)
            nc.vector.tensor_copy(out=G[:, t, GW:GW + OHW], in_=oh_ps[:])

        # zero the pad columns so stray NaNs never appear downstream
        nc.vector.memset(G[:, :, GW + OHW:], 0.0)

        # ---------------- squares for 2nd-order FM
        E2 = big.tile([P, NT, GBLK], F32)
        for j in range(4):
            sl = slice(j * NT // 4, (j + 1) * NT // 4)
            nc.vector.tensor_tensor(out=E2[:, sl, :], in0=G[:, sl, :],
                                    in1=G[:, sl, :], op=mybir.AluOpType.mult)

        fmw = ctx.enter_context(tc.tile_pool(name="fmw", bufs=1))
        # s = sum_f E_f  (batch-major [128, 16, 32])
        s_bm = fmw.tile([P, NT, EMB], F32)
        q_bm = fmw.tile([P, NT, EMB], F32)
        tmp = fmw.tile([P, NT, EMB], F32)
        for (src, dst) in ((G, s_bm), (E2, q_bm)):
            # gathered fields: [p, t, e, f(stride 33)] reduce f
            g_ef = src[:, :, 0:GW].rearrange("p t (f e) -> p t f e", e=D)
            g_ef = g_ef[:, :, :, 0:EMB].rearrange("p t f e -> p t e f")
            nc.vector.tensor_reduce(out=dst[:], in_=g_ef,
                                    axis=mybir.AxisListType.X, op=mybir.AluOpType.add)
            # onehot fields: [p, t, e, j(stride 32)] reduce j
            o_ef = src[:, :, GW:GW + 96].rearrange("p t (j e) -> p t j e", e=EMB)
            o_ef = o_ef.rearrange("p t j e -> p t e j")
            nc.vector.tensor_reduce(out=tmp[:], in_=o_ef,
                                    axis=mybir.AxisListType.X, op=mybir.AluOpType.add)
            nc.vector.tensor_add(out=dst[:], in0=dst[:], in1=tmp[:])

        # fm2 = 0.5 * sum_e (s^2 - q)
        nc.vector.tensor_tensor(out=s_bm[:], in0=s_bm[:], in1=s_bm[:],
                                op=mybir.AluOpType.mult)
        nc.vector.tensor_sub(out=s_bm[:], in0=s_bm[:], in1=q_bm[:])
        fm_bm = fmw.tile([P, NT], F32)
        nc.vector.tensor_reduce(out=fm_bm[:], in_=s_bm[:],
                                axis=mybir.AxisListType.X, op=mybir.AluOpType.add)
        nc.vector.tensor_scalar_mul(fm_bm[:], fm_bm[:], 0.5)

        # fm1: gathered fm1 cols + onehot/dense col + const bias
        fm1_g = fmw.tile([P, NT], F32)
        g_f1 = G[:, :, 0:GW].rearrange("p t (f e) -> p t f e", e=D)[:, :, :, EMB:EMB + 1]
        g_f1 = g_f1.rearrange("p t f e -> p t e f")
        nc.vector.tensor_reduce(out=fm1_g[:], in_=g_f1,
                                axis=mybir.AxisListType.X, op=mybir.AluOpType.add)
        nc.vector.tensor_add(out=fm_bm[:], in0=fm_bm[:], in1=fm1_g[:])
        nc.vector.tensor_add(out=fm_bm[:], in0=fm_bm[:], in1=G[:, :, GW + 96])
        nc.vector.tensor_scalar_add(fm_bm[:], fm_bm[:], cb_t[:])

        # ---------------- feature-major dnn input: ET[k][:, n] via PE transposes
        ET = [big.tile([KSIZES[k], BC], F32, name=f"ett{k}", tag=f"et_{k}") for k in range(NK)]
        for n in range(NN):
            for k in range(NK):
                tr_ps = ps_tr.tile([P, NB], F32, tag=f"tr")
                for j in range(4):
                    t = n * 4 + j
                    nc.tensor.matmul(
                        tr_ps[:KSIZES[k], j * P:(j + 1) * P],
                        lhsT=G[:, t, KOFF[k]:KOFF[k] + KSIZES[k]],
                        rhs=ident[:], start=True, stop=True)
                nc.vector.tensor_copy(out=ET[k][:, n * NB:(n + 1) * NB],
                                      in_=tr_ps[:KSIZES[k], :])
            # dense relu term for this n-chunk, added into ET
            for k in range(NK):
                r_ps = ps_mm.tile([P, NB], F32, name="rps", tag="mm")
                nc.tensor.matmul(
                    r_ps[:KSIZES[k], :],
                    lhsT=dwp_t[:, KOFF[k]:KOFF[k] + KSIZES[k]],
                    rhs=dft_t[:, n * NB:(n + 1) * NB], start=True, stop=True)
                rr = work.tile([P, NB], F32, tag="rr")
                nc.scalar.activation(out=rr[:KSIZES[k], :], in_=r_ps[:KSIZES[k], :],
                                     func=mybir.ActivationFunctionType.Relu,
                                     bias=dbp_t[k][:], scale=1.0)
                nc.vector.tensor_add(out=ET[k][:, n * NB:(n + 1) * NB],
                                     in0=ET[k][:, n * NB:(n + 1) * NB],
                                     in1=rr[:KSIZES[k], :])

        # ---------------- MLP feature-major
        h0 = [big.tile([P, BC], F32, name=f"h0t{m}", tag=f"h0_{m}") for m in range(2)]
        for n in range(NN):
            for m in range(2):
                ps = ps_mm.tile([P, NB], F32, name="h0ps", tag="mm")
                for k in range(NK):
                    nc.tensor.matmul(ps[:], lhsT=w0_t[k][:, m * P:(m + 1) * P],
                                     rhs=ET[k][:, n * NB:(n + 1) * NB],
                                     start=(k == 0), stop=(k == NK - 1))
                nc.scalar.activation(out=h0[m][:, n * NB:(n + 1) * NB], in_=ps[:],
                                     func=mybir.ActivationFunctionType.Relu,
                                     bias=b0_t[:, m:m + 1], scale=1.0)
        h1 = big.tile([P, BC], F32)
        for n in range(NN):
            ps = ps_mm.tile([P, NB], F32, name="h1ps", tag="mm")
            for m in range(2):
                nc.tensor.matmul(ps[:], lhsT=w1_t[m][:],
                                 rhs=h0[m][:, n * NB:(n + 1) * NB],
                                 start=(m == 0), stop=(m == 1))
            nc.scalar.activation(out=h1[:, n * NB:(n + 1) * NB], in_=ps[:],
                                 func=mybir.ActivationFunctionType.Relu,
                                 bias=b1_t[:], scale=1.0)

        # dnn_out batch-major: per tile matmul h1T-chunk.T @ w2 -> [128, 1]
        dn_ps = ps_dn.tile([P, NT], F32, name="dnps", tag="dn")
        for t in range(NT):
            nc.tensor.matmul(dn_ps[:, t:t + 1], lhsT=h1[:, t * P:(t + 1) * P],
                             rhs=w2_t[:], start=True, stop=True)
        outv = work.tile([P, NT], F32, tag="outv")
        nc.vector.tensor_add(out=outv[:], in0=fm_bm[:], in1=dn_ps[:])

        # transpose [128, 16] -> [16, 128] and store
        fin_ps = ps_dn.tile([NT, P], F32, name="finps", tag="dn")
        nc.tensor.matmul(fin_ps[:], lhsT=outv[:], rhs=ident[:], start=True, stop=True)
        fin = work.tile([NT, P], F32, tag="fin")
        nc.vector.tensor_copy(out=fin[:], in_=fin_ps[:])
        nc.sync.dma_start(out=out.rearrange("(t p) -> t p", p=P), in_=fin[:])


# ---------------------------------------------------------------- host side

def _prep_host(inputs):
    """Build per-core input maps from the full problem inputs."""
    f32 = np.float32
    item_features = np.asarray(inputs["item_features"], f32)
    user_features = np.asarray(inputs["user_features"], f32)
    user_id = np.asarray(inputs["user_id"]).astype(np.int64)
    target_item_id = np.asarray(inputs["target_item_id"]).astype(np.int64)
    item_fm1 = [np.asarray(a, f32) for a in inputs["item_fm1"]]
    user_fm1 = [np.asarray(a, f32) for a in inputs["user_fm1"]]
    item_fm2 = [np.asarray(a, f32) for a in inputs["item_fm2"]]
    user_fm2 = [np.asarray(a, f32) for a in inputs["user_fm2"]]
    item_emb = np.asarray(inputs["item_emb_table"], f32)
    user_emb = np.asarray(inputs["user_emb_table"], f32)

    # combined table [VTOT, 33]
    tbl = np.zeros((VTOT, D), f32)
    parts = (
        [(item_fm2[i], item_fm1[i]) for i in range(4)]
        + [(user_fm2[i], user_fm1[i]) for i in range(3)]
        + [(item_emb, None), (user_emb, None)]
    )
    for f, (t2, t1) in enumerate(parts):
        b = BASES[f]
        tbl[b:b + t2.shape[0], :EMB] = t2
        if t1 is not None:
            tbl[b:b + t1.shape[0], EMB] = t1[:, 0]

    # global row indices [B, 9]
    gidx = np.empty((B, NF), np.int64)
    for j, c in enumerate(ITEM_CATE_COLS):
        gidx[:, j] = item_features[:, c].astype(np.int64) + BASES[j]
    for j, c in enumerate(USER_CATE_COLS):
        gidx[:, 4 + j] = user_features[:, c].astype(np.int64) + BASES[4 + j]
    gidx[:, 7] = target_item_id + BASES[7]
    gidx[:, 8] = user_id + BASES[8]
    gidx = gidx.astype(np.int32)

    # onehot inputs + dense feat, transposed [46, B]
    x_oh = np.concatenate([
        item_features[:, 8:18].astype(np.int32).astype(f32),
        item_features[:, 18:38].astype(np.int32).astype(f32),
        user_features[:, 5:15].astype(np.int32).astype(f32),
    ], axis=1)                                   # [B, 40]
    dense = np.concatenate([item_features[:, 0:4], user_features[:, 0:2]], axis=1)
    xd = np.concatenate([x_oh, dense], axis=1)   # [B, 46]

    # wcat [46, 97]
    wcat = np.zeros((46, OHW), f32)
    wcat[0:10, 0:32] = item_fm2[4][1:11]
    wcat[10:30, 32:64] = item_fm2[5][1:21]
    wcat[30:40, 64:96] = user_fm2[3][1:11]
    wcat[0:10, 96] = item_fm1[4][1:11, 0]
    wcat[10:30, 96] = item_fm1[5][1:21, 0]
    wcat[30:40, 96] = user_fm1[3][1:11, 0]
    wcat[40:46, 96] = np.asarray(inputs["fm1_dense_W"], f32)[:, 0]

    inv = f32(1.0 / np.sqrt(1.0 + EPS))
    g0 = np.asarray(inputs["g0"], f32)
    be0 = np.asarray(inputs["be0"], f32)
    g1 = np.asarray(inputs["g1"], f32)
    be1 = np.asarray(inputs["be1"], f32)
    W0 = np.asarray(inputs["W0"], f32) * (inv * g0)[None, :]
    b0 = np.asarray(inputs["b0"], f32) * (inv * g0) + be0
    W1 = np.asarray(inputs["W1"], f32) * (inv * g1)[None, :]
    b1 = np.asarray(inputs["b1"], f32) * (inv * g1) + be1

    # W0 permuted into the 393-row device layout
    w0p = np.zeros((393, 256), f32)
    for f in range(NF):
        df = DNN_FIELD_OF_GATHER[f]
        w0p[f * D:f * D + EMB, :] = W0[df * EMB:(df + 1) * EMB, :]
    for j, df in enumerate(DNN_FIELD_OF_OH):
        w0p[GW + j * EMB:GW + (j + 1) * EMB, :] = W0[df * EMB:(df + 1) * EMB, :]

    dense_W = np.asarray(inputs["dense_W"], f32)   # [6, 384]
    dense_b = np.asarray(inputs["dense_b"], f32)   # [384]
    dwp = np.zeros((6, 393), f32)
    dbp = np.zeros((393, 1), f32)
    for f in range(NF):
        df = DNN_FIELD_OF_GATHER[f]
        dwp[:, f * D:f * D + EMB] = dense_W[:, df * EMB:(df + 1) * EMB]
        dbp[f * D:f * D + EMB, 0] = dense_b[df * EMB:(df + 1) * EMB]
    for j, df in enumerate(DNN_FIELD_OF_OH):
        dwp[:, GW + j * EMB:GW + (j + 1) * EMB] = dense_W[:, df * EMB:(df + 1) * EMB]
        dbp[GW + j * EMB:GW + (j + 1) * EMB, 0] = dense_b[df * EMB:(df + 1) * EMB]

    cbv = np.asarray(inputs["fm1_dense_b"], f32).reshape(1, 1)
    w2v = np.asarray(inputs["W2"], f32)

    in_maps = []
    for c in range(NCORES):
        sl = slice(c * BC, (c + 1) * BC)
        gi = gidx[sl].reshape(NT, P, NF).transpose(1, 0, 2).reshape(P, NT * NF)
        in_maps.append({
            "tbl": tbl,
            "idx": np.ascontiguousarray(gi),
            "xdt": np.ascontiguousarray(xd[sl].T),
            "dft": np.ascontiguousarray(dense[sl].T),
            "wcat": wcat,
            "w0p": w0p,
            "dwp": dwp,
            "dbp": dbp,
            "b0f": np.ascontiguousarray(b0.reshape(256, 1)),
            "w1p": W1,
            "b1f": np.ascontiguousarray(b1.reshape(128, 1)),
            "w2": w2v,
            "cb": cbv,
        })
    return in_maps


def kernel(**inputs) -> np.ndarray:
    if "nc" not in _cache:
        _cache["nc"] = _build()
    nc = _cache["nc"]
    in_maps = _prep_host(inputs)
    res = bass_utils.run_bass_kernel_spmd(nc, in_maps, core_ids=list(range(NCORES)))
    out = np.concatenate([res.results[c]["out"] for c in range(NCORES)])
    return out.reshape(B, 1).astype(np.float32)
